# revision 3
# baseline (speedup 1.0000x reference)
"""Dense transformer block (attention + post-LN MLP) on 8 trn2 NeuronCores.

Context-parallel sharding: core c handles batch c//4 and token blocks
j*256, (7-j)*256 of that batch (j = c%4) -> causally balanced 512
tokens/core. Weights replicated (bf16). One AllGather of K/V per
4-core batch group. Activations feature-major (x^T) end-to-end.
"""

import numpy as np
import ml_dtypes

BF16 = ml_dtypes.bfloat16

N_CORES = 8
B, S, D = 2, 2048, 1024
H, HD = 16, 64
F = 4 * D
TOK = 512            # tokens per core
P = 128
NKC = S // P         # 16 key chunks per batch
NHP = H // 2         # 8 head pairs
MASK_NEG = -80000.0  # -> -79872 in bf16; /8 => exp underflows to exactly 0
LN_EPS = 1e-5

_CACHE = {}


def _q_blocks(j):
    """Global 128-blocks of this core's query tokens, ascending."""
    return [2 * j, 2 * j + 1, 14 - 2 * j, 15 - 2 * j]


def _build():
    import concourse.bass as bass
    import concourse.mybir as mybir
    import concourse.tile as tile
    from concourse import bacc

    dt = mybir.dt
    AF = mybir.ActivationFunctionType
    OP = mybir.AluOpType

    nc = bacc.Bacc(
        "TRN2",
        target_bir_lowering=False,
        debug=False,
        enable_asserts=True,
        num_devices=N_CORES,
    )

    def din(name, shape, dty):
        return nc.dram_tensor(name, shape, dty, kind="ExternalInput").ap()

    xT_f32 = din("xT_f32", [D, TOK], dt.float32)
    xT_bf = din("xT_bf", [D, TOK], dt.bfloat16)
    w_qk = din("w_qk", [D, 2 * D], dt.bfloat16)
    w_v = din("w_v", [D, D], dt.bfloat16)
    w_o = din("w_o", [D, D], dt.bfloat16)
    w_fc = din("w_fc", [D, F], dt.bfloat16)
    w_pr = din("w_pr", [F, D], dt.bfloat16)
    b_qk = din("b_qk", [2 * D], dt.float32)
    b_o = din("b_o", [D], dt.float32)
    b_fc = din("b_fc", [F], dt.float32)
    b_pr = din("b_pr", [D], dt.float32)
    g1 = din("g1", [D], dt.float32)
    b1 = din("b1", [D], dt.float32)
    g2 = din("g2", [D], dt.float32)
    b2 = din("b2", [D], dt.float32)
    ident_d = din("ident", [P, P], dt.bfloat16)
    maskm_d = din("maskm", [P, P], dt.bfloat16)
    jidx_d = din("jidx", [1, 1], dt.uint32)
    out_d = nc.dram_tensor("out", [D, TOK], dt.float32, kind="ExternalOutput").ap()

    VW = 66  # per-head V row width: 64 v + ones col (64) + pad (65)

    with tile.TileContext(nc) as tc:
        from contextlib import ExitStack

        ctx = ExitStack()
        with ctx:
            c_pool = ctx.enter_context(tc.tile_pool(name="consts", bufs=1))
            dram = ctx.enter_context(tc.tile_pool(name="dram", bufs=1, space="DRAM"))

            # ---- per-core j register (for Switch) ----
            jreg = nc.alloc_registers(
                "jreg",
                [mybir.EngineType.PE, mybir.EngineType.Activation,
                 mybir.EngineType.DVE, mybir.EngineType.SP],
            )
            nc.regs_load(jreg, jidx_d[0:1, 0:1])
            jval = nc.snap(jreg, donate=True, min_val=0, max_val=3)

            # ---- constants ----
            ident = c_pool.tile([P, P], dt.bfloat16, tag="ident")
            nc.sync.dma_start(ident[:], ident_d[:])
            maskm = c_pool.tile([P, P], dt.bfloat16, tag="maskm")
            nc.sync.dma_start(maskm[:], maskm_d[:])
            ones128_bf = c_pool.tile([P, 1], dt.bfloat16, tag="ones128")
            nc.vector.memset(ones128_bf[:], 1.0)
            ones1_f = c_pool.tile([1, P], dt.float32, tag="ones1f")
            nc.vector.memset(ones1_f[:], 1.0)
            ones_hi = c_pool.tile([65, 64], dt.bfloat16, tag="oneshi")
            nc.vector.memset(ones_hi[64:65, :], 1.0)

            bqk_sb = c_pool.tile([P, 16], dt.float32, tag="bqk")
            nc.sync.dma_start(bqk_sb[:], b_qk.rearrange("(c p) -> p c", p=P))
            bo_sb = c_pool.tile([P, 8], dt.float32, tag="bo")
            nc.sync.dma_start(bo_sb[:], b_o.rearrange("(c p) -> p c", p=P))
            bfc_sb = c_pool.tile([P, 32], dt.float32, tag="bfc")
            nc.sync.dma_start(bfc_sb[:], b_fc.rearrange("(c p) -> p c", p=P))
            bpr_sb = c_pool.tile([P, 8], dt.float32, tag="bpr")
            nc.sync.dma_start(bpr_sb[:], b_pr.rearrange("(c p) -> p c", p=P))
            g1_sb = c_pool.tile([P, 8], dt.float32, tag="g1")
            nc.sync.dma_start(g1_sb[:], g1.rearrange("(c p) -> p c", p=P))
            b1_sb = c_pool.tile([P, 8], dt.float32, tag="b1")
            nc.sync.dma_start(b1_sb[:], b1.rearrange("(c p) -> p c", p=P))
            g2_sb = c_pool.tile([P, 8], dt.float32, tag="g2")
            nc.sync.dma_start(g2_sb[:], g2.rearrange("(c p) -> p c", p=P))
            b2_sb = c_pool.tile([P, 8], dt.float32, tag="b2")
            nc.sync.dma_start(b2_sb[:], b2.rearrange("(c p) -> p c", p=P))

            # ---- AG buffers: 4 sub-1MB collectives (mesh algo regime) ----
            KH = 4 * P * TOK                 # half of kT  [4,128,512]
            VH = 2 * P * (16 * VW)           # half of v   [2,128,1056]
            ag_k_in = [dram.tile([KH], dt.bfloat16, tag=f"agki{i}", name=f"agki{i}") for i in range(2)]
            ag_k_out = [dram.tile([4, KH], dt.bfloat16, tag=f"agko{i}", name=f"agko{i}") for i in range(2)]
            ag_v_in = [dram.tile([VH], dt.bfloat16, tag=f"agvi{i}", name=f"agvi{i}") for i in range(2)]
            ag_v_out = [dram.tile([4, VH], dt.bfloat16, tag=f"agvo{i}", name=f"agvo{i}") for i in range(2)]
            GROUPS = [[0, 1, 2, 3], [4, 5, 6, 7]]

            # ---- r1 outermost (spans w_o..LN1; stack discipline) ----
            pR_cm = tc.tile_pool(name="pR", bufs=1)
            pR = pR_cm.__enter__()
            r1 = pR.tile([P, 8, TOK], dt.float32, tag="r1")

            # ---- qT / o_cat / xfs / wo live from phase 1 through w_o ----
            pA_cm = tc.tile_pool(name="pA", bufs=1)
            pA = pA_cm.__enter__()
            qT = pA.tile([P, 8, TOK], dt.bfloat16, tag="qT")
            o_cat = [pA.tile([P, TOK], dt.bfloat16, tag=f"o_cat{i}", name=f"o_cat{i}")
                     for i in range(NHP)]
            o_tmp = [pA.tile([64, TOK], dt.bfloat16, tag=f"o_tmp{i}", name=f"o_tmp{i}") for i in range(NHP)]
            wo_sb = pA.tile([P, 8, D], dt.bfloat16, tag="wo")

            # gathered K/V + own-kv pools open across phase1+attention
            gat_cm = tc.tile_pool(name="gathered", bufs=1)
            gat = gat_cm.__enter__()
            kT_pack = gat.tile([P, 8, S], dt.bfloat16, tag="kT_pack")
            v_ones = gat.tile([P, NKC, 16 * VW], dt.bfloat16, tag="v_ones")
            kvp_cm = tc.tile_pool(name="kv_own", bufs=1)
            kvp = kvp_cm.__enter__()
            v_own = kvp.tile([P, 4, 16 * VW], dt.bfloat16, tag="v_own")
            kT_own = kvp.tile([P, 8, TOK], dt.bfloat16, tag="kT_own")

            # ============ phase 1: qkv ============
            with (
                tc.tile_pool(name="xw", bufs=2) as xw,
                tc.tile_pool(name="qkv_ps", bufs=4, space="PSUM") as qkv_ps,
            ):
                xb = xw.tile([P, 8, TOK], dt.bfloat16, tag="xb")
                xbv = xT_bf.rearrange("(c p) t -> p c t", p=P)
                wv = xw.tile([P, 8, D], dt.bfloat16, tag="wv")
                wvv = w_v.rearrange("(c p) f -> p c f", p=P)
                for h in range(4):
                    nc.sync.dma_start(xb[:, 2 * h:2 * h + 2, :], xbv[:, 2 * h:2 * h + 2, :])
                    nc.sync.dma_start(wv[:, 2 * h:2 * h + 2, :], wvv[:, 2 * h:2 * h + 2, :])

                v4 = v_own.rearrange("p c (h w) -> p c h w", w=VW)
                nc.vector.memset(v4[:, :, :, 64:66], 0.0)
                nc.vector.memset(v4[:, :, :, 64:65], 1.0)

                def v_block(t4):
                    for vc in range(2):
                        ps = qkv_ps.tile([P, 512], dt.float32, tag="qkvps")
                        for c8 in range(8):
                            nc.tensor.matmul(
                                ps[:],
                                lhsT=xb[:, c8, t4 * P:(t4 + 1) * P],
                                rhs=wv[:, c8, vc * 512:(vc + 1) * 512],
                                start=(c8 == 0),
                                stop=(c8 == 7),
                            )
                        dest = v4[:, t4, vc * 8:(vc + 1) * 8, 0:64]
                        nc.vector.tensor_copy(dest, ps[:].rearrange("p (h w) -> p h w", w=64))

                def ship_v(i):
                    nc.sync.dma_start(
                        ag_v_in[i].rearrange("(c p f) -> p c f", c=2, p=P),
                        v_own[:, 2 * i:2 * i + 2, :],
                    )
                    nc.gpsimd.collective_compute(
                        "AllGather", mybir.AluOpType.bypass,
                        replica_groups=GROUPS,
                        ins=[ag_v_in[i].opt()], outs=[ag_v_out[i].opt()],
                    )

                def qk_quarter(qi):
                    # w_qk columns: [q(0:1024) | k(1024:2048)]
                    wq = xw.tile([P, 8, 512], dt.bfloat16, tag="wqkq")
                    nc.sync.dma_start(
                        wq[:],
                        w_qk[:, qi * 512:(qi + 1) * 512].rearrange(
                            "(c p) f -> p c f", p=P),
                    )
                    for fo in range(4):
                        fchunk = 4 * qi + fo
                        ps = qkv_ps.tile([P, 512], dt.float32, tag="qkvps")
                        for c8 in range(8):
                            nc.tensor.matmul(
                                ps[:],
                                lhsT=wq[:, c8, fo * P:(fo + 1) * P],
                                rhs=xb[:, c8, :],
                                start=(c8 == 0),
                                stop=(c8 == 7),
                            )
                        dest = kT_own[:, fchunk - 8, :] if fchunk >= 8 else qT[:, fchunk, :]
                        nc.scalar.activation(
                            dest, ps[:], AF.Identity,
                            bias=bqk_sb[:, fchunk:fchunk + 1],
                        )
                    if qi in (2, 3):  # ship kT half as soon as evicted
                        i = qi - 2
                        nc.sync.dma_start(
                            ag_k_in[i].rearrange("(c p t) -> p c t", c=4, p=P),
                            kT_own[:, 4 * i:4 * i + 4, :],
                        )
                        nc.gpsimd.collective_compute(
                            "AllGather", mybir.AluOpType.bypass,
                            replica_groups=GROUPS,
                            ins=[ag_k_in[i].opt()], outs=[ag_k_out[i].opt()],
                        )

                # order: v01 -> AGv0 -> k quarters (AGk0, AGk1) -> v23 ->
                # AGv1 -> q quarters.  Pass-2 attention needs k0+v0 first.
                v_block(0)
                v_block(1)
                ship_v(0)
                qk_quarter(2)
                v_block(2)
                v_block(3)
                ship_v(1)
                qk_quarter(3)
                qk_quarter(0)
                qk_quarter(1)

            # prefetch (DMA engines, overlaps attention)
            nc.sync.dma_start(wo_sb[:], w_o.rearrange("(c p) f -> p c f", p=P))

            # ============ phase 2+3: K/V load + attention ============
            with (
                tc.tile_pool(name="att_s", bufs=2, space="PSUM") as s_ps_pool,
                tc.tile_pool(name="att_o", bufs=1, space="PSUM") as o_ps_pool,
                tc.tile_pool(name="att_rep", bufs=2, space="PSUM") as rep_ps_pool,
                tc.tile_pool(name="att_tmp", bufs=3) as atmp,
                tc.tile_pool(name="att_tmp2", bufs=2) as atmp2,
                tc.tile_pool(name="att_loc", bufs=1) as alocp,
            ):
                def _load_k(r, i, j):
                    if r == j:
                        kk = kT_own[:, 4 * i:4 * i + 4, :]
                    else:
                        kk = ag_k_out[i][r].rearrange("(c p t) -> p c t", c=4, p=P)
                    for hc in range(4):
                        hp = 4 * i + hc
                        nc.sync.dma_start(
                            kT_pack[:, hp, r * 256:(r + 1) * 256], kk[:, hc, 0:256])
                        nc.sync.dma_start(
                            kT_pack[:, hp, (7 - r) * 256:(8 - r) * 256],
                            kk[:, hc, 256:512])

                def _load_v(r, i, j):
                    if r == j:
                        vv = v_own[:, 2 * i:2 * i + 2, :]
                    else:
                        vv = ag_v_out[i][r].rearrange("(c p f) -> p c f", c=2, p=P)
                    for lc in range(2):
                        lt = 2 * i + lc
                        gb = r if lt < 2 else 7 - r
                        g16 = 2 * gb + (lt % 2)
                        nc.sync.dma_start(v_ones[:, g16, :], vv[:, lc, :])

                def load_kv(j):
                    """Fill kT_pack / v_ones.  Own rank straight from SBUF
                    (overlaps the AG chain); remote halves in AG completion
                    order (v0, k0, k1, v1) so the DMA queue never blocks an
                    available load behind a pending collective."""
                    for i in range(2):
                        _load_k(j, i, j)
                        _load_v(j, i, j)
                    rem = [r for r in range(4) if r != j]
                    for r in rem:
                        _load_v(r, 0, j)
                    for r in rem:
                        _load_k(r, 0, j)
                    for r in rem:
                        _load_v(r, 1, j)
                    for r in rem:
                        _load_k(r, 1, j)

                # ============ phase 3: attention (per-core causal variant) ============
                def attn_qkav(hp, kc, gq, oT, first, last):
                    vi = [i for i in range(4) if gq[i] >= kc]
                    qs = P * vi[0]
                    diag = gq[vi[0]] == kc
                    sps = s_ps_pool.tile([P, 2, TOK], dt.float32, tag="sps")
                    for h2 in range(2):
                        lo, hi = 64 * h2, 64 * (h2 + 1)
                        nc.tensor.matmul(
                            sps[:, h2, qs:],
                            lhsT=kT_pack[lo:hi, hp, kc * P:(kc + 1) * P],
                            rhs=qT[lo:hi, hp, qs:],
                            start=True,
                            stop=not diag,
                            tile_position=(64 * h2, 0),
                        )
                        if diag:
                            nc.tensor.matmul(
                                sps[:, h2, qs:qs + P],
                                lhsT=ident[:],
                                rhs=maskm[:],
                                start=False,
                                stop=True,
                            )
                    aT = atmp.tile([P, 2, TOK], dt.bfloat16, tag="aT")
                    nc.scalar.activation(
                        aT[:, :, qs:], sps[:, :, qs:], AF.Exp, scale=0.125
                    )
                    for h2 in range(2):
                        h = 2 * hp + h2
                        nc.tensor.matmul(
                            oT[:, h2, qs:],
                            lhsT=v_ones[:, kc, h * VW:h * VW + 65],
                            rhs=aT[:, h2, qs:],
                            start=first,
                            stop=last,
                        )

                def attn_arm(j):
                    load_kv(j)
                    gq = _q_blocks(j)
                    last_kc = gq[-1]
                    remote = [k for k in range(last_kc + 1) if k not in gq]
                    # pass 1: all local chunks (overlap the AllGather chain)
                    oloc = []
                    for hp in range(NHP):
                        oT = o_ps_pool.tile([65, 2, TOK], dt.float32, tag="oT")
                        for ki, kc in enumerate(gq):
                            attn_qkav(hp, kc, gq, oT, ki == 0, kc == gq[-1])
                        ol = alocp.tile([65, 2, TOK], dt.float32,
                                       tag=f"oloc{hp}", name=f"ol{hp}")
                        nc.vector.tensor_copy(ol[:], oT[:])
                        oloc.append(ol)
                    # pass 2: remote chunks + local partial + normalize
                    qs0 = P * [i for i in range(4) if gq[i] >= remote[0]][0]
                    for hp in range(NHP):
                        oT = o_ps_pool.tile([65, 2, TOK], dt.float32, tag="oT")
                        if qs0 > 0:
                            # remote pass never writes cols [0:qs0); zero them
                            # so the local+remote add reads 0, not stale psum
                            nc.vector.memset(oT[:, :, 0:qs0], 0.0)
                        for ki, kc in enumerate(remote):
                            attn_qkav(hp, kc, gq, oT, ki == 0, ki == len(remote) - 1)
                        osum = atmp2.tile([65, 2, TOK], dt.float32, tag="osum")
                        nc.vector.tensor_tensor(osum[:], oT[:], oloc[hp][:], OP.add)
                        # softmax denominators -> reciprocals via ln/exp
                        lnrow = atmp2.tile([65, 2, TOK], dt.float32, tag="lnrow")
                        nc.scalar.activation(
                            lnrow[64:65, :, :], osum[64:65, :, :], AF.Ln
                        )
                        rrow = atmp2.tile([65, 2, TOK], dt.bfloat16, tag="rrow")
                        nc.scalar.activation(
                            rrow[64:65, :, :], lnrow[64:65, :, :], AF.Exp, scale=-1.0
                        )
                        for h2 in range(2):
                            rep = rep_ps_pool.tile([64, TOK], dt.float32, tag="rep")
                            nc.tensor.matmul(
                                rep[:],
                                lhsT=ones_hi[64:65, :],
                                rhs=rrow[64:65, h2, :],
                                start=True,
                                stop=True,
                            )
                            rep_sb = atmp2.tile([64, TOK], dt.float32, tag="rep_sb")
                            nc.scalar.activation(rep_sb[:], rep[:], AF.Copy)
                            dest = o_cat[hp][0:64, :] if h2 == 0 else o_tmp[hp][:]
                            nc.vector.tensor_tensor(
                                dest, osum[0:64, h2, :], rep_sb[:], OP.mult
                            )
                        nc.sync.dma_start(o_cat[hp][64:128, :], o_tmp[hp][:])

                for case in tc.Switch(jval, 4):
                    attn_arm(case)

            kvp_cm.__exit__(None, None, None)
            gat_cm.__exit__(None, None, None)

            # ============ phase 4: w_o + residual ============
            pX2_cm = tc.tile_pool(name="pX2", bufs=1)
            pX2 = pX2_cm.__enter__()
            xfs = pX2.tile([P, 8, TOK], dt.float32, tag="xfs")
            nc.sync.dma_start(xfs[:], xT_f32.rearrange("(c p) t -> p c t", p=P))
            with tc.tile_pool(name="wo_ps", bufs=4, space="PSUM") as wo_ps:
                for oc in range(8):
                    ps = wo_ps.tile([P, 512], dt.float32, tag="wops")
                    for hp in range(8):
                        nc.tensor.matmul(
                            ps[:],
                            lhsT=wo_sb[:, hp, oc * P:(oc + 1) * P],
                            rhs=o_cat[hp][:],
                            start=(hp == 0),
                            stop=(hp == 7),
                        )
                    nc.vector.scalar_tensor_tensor(
                        r1[:, oc, :], ps[:], bo_sb[:, oc:oc + 1], xfs[:, oc, :],
                        op0=OP.add, op1=OP.add,
                    )

            # ============ layernorm helper (feature-major, partition reduce) ====
            def layernorm(src, dst, g_sb, b_sb, tag):
                with (
                    tc.tile_pool(name=f"ln_{tag}", bufs=2) as lnp,
                    tc.tile_pool(name=f"lnps_{tag}", bufs=2, space="PSUM") as lnps,
                    tc.tile_pool(name=f"lnrep_{tag}", bufs=1, space="PSUM") as lnrep,
                ):
                    src_bf = lnp.tile([P, 8, TOK], dt.bfloat16, tag=f"srcbf_{tag}")
                    for c8 in range(8):
                        nc.vector.tensor_copy(src_bf[:, c8, :], src[:, c8, :])
                    s1 = lnps.tile([1, TOK], dt.float32, tag=f"s1_{tag}")
                    s2 = lnps.tile([1, TOK], dt.float32, tag=f"s2_{tag}")
                    for c8 in range(8):
                        sq = lnp.tile([P, TOK], dt.bfloat16, tag=f"sq_{tag}")
                        nc.vector.tensor_tensor(sq[:], src[:, c8, :], src[:, c8, :], OP.mult)
                        nc.tensor.matmul(
                            s1[:], lhsT=ones128_bf[:], rhs=src_bf[:, c8, :],
                            start=(c8 == 0), stop=(c8 == 7),
                        )
                        nc.tensor.matmul(
                            s2[:], lhsT=ones128_bf[:], rhs=sq[:],
                            start=(c8 == 0), stop=(c8 == 7),
                        )
                    mu = lnp.tile([1, TOK], dt.float32, tag=f"mu_{tag}")
                    nc.vector.tensor_scalar(mu[:], s1[:], 1.0 / D, None, OP.mult)
                    m2 = lnp.tile([1, TOK], dt.float32, tag=f"m2_{tag}")
                    # fold the LN eps in here: m2 = s2/D + eps
                    nc.vector.tensor_scalar(m2[:], s2[:], 1.0 / D, LN_EPS, OP.mult, OP.add)
                    var = lnp.tile([1, TOK], dt.float32, tag=f"var_{tag}")
                    nc.vector.tensor_tensor(var[:], mu[:], mu[:], OP.mult)
                    nc.vector.tensor_tensor(var[:], m2[:], var[:], OP.subtract)
                    lnv = lnp.tile([1, TOK], dt.float32, tag=f"lnv_{tag}")
                    nc.scalar.activation(lnv[:], var[:], AF.Ln)
                    rsmu = lnp.tile([1, 2, TOK], dt.float32, tag=f"rsmu_{tag}")
                    nc.scalar.activation(rsmu[:, 0, :], lnv[:], AF.Exp, scale=-0.5)
                    nc.vector.tensor_tensor(
                        rsmu[:, 1, :], mu[:], rsmu[:, 0, :], OP.mult
                    )
                    rep = lnrep.tile([P, 2, TOK], dt.float32, tag=f"rep_{tag}")
                    for i in range(2):
                        nc.tensor.matmul(
                            rep[:, i, :], lhsT=ones1_f[:], rhs=rsmu[:, i, :],
                            start=True, stop=True,
                        )
                    rep_sb = lnp.tile([P, 2, TOK], dt.float32, tag=f"repsb_{tag}")
                    nc.scalar.activation(rep_sb[:], rep[:], AF.Copy)
                    for c8 in range(8):
                        t = lnp.tile([P, TOK], dt.float32, tag=f"t_{tag}")
                        nc.vector.tensor_tensor(
                            t[:], src[:, c8, :], rep_sb[:, 0, :], OP.mult
                        )
                        nc.vector.tensor_tensor(t[:], t[:], rep_sb[:, 1, :], OP.subtract)
                        nc.vector.tensor_scalar(
                            dst[:, c8, :], t[:], g_sb[:, c8:c8 + 1], b_sb[:, c8:c8 + 1],
                            OP.mult, OP.add,
                        )

            pX2_cm.__exit__(None, None, None)
            pA_cm.__exit__(None, None, None)
            pN_cm = tc.tile_pool(name="pN", bufs=1)
            pN = pN_cm.__enter__()
            n_sb = pN.tile([P, 8, TOK], dt.float32, tag="n_sb")
            n_bf = [pN.tile([P, TOK], dt.bfloat16, tag=f"n_bf{i}", name=f"n_bf{i}")
                    for i in range(8)]
            h1g = pN.tile([P, 32, TOK], dt.bfloat16, tag="h1g")
            r2 = pN.tile([P, 8, TOK], dt.float32, tag="r2")

            layernorm(r1, n_sb, g1_sb, b1_sb, "ln1")
            for c8 in range(8):
                nc.vector.tensor_copy(n_bf[c8][:], n_sb[:, c8, :])

            # ============ phase 5: MLP ============
            wprp_cm = tc.tile_pool(name="wpr", bufs=2)
            wprp = wprp_cm.__enter__()

            def load_wpr(q4):
                wq = wprp.tile([P, 8, D], dt.bfloat16, tag="wprq")
                nc.sync.dma_start(
                    wq[:],
                    w_pr[q4 * 1024:(q4 + 1) * 1024, :].rearrange(
                        "(c p) f -> p c f", p=P),
                )
                return wq

            wpr_q0 = load_wpr(0)  # prefetch during fc

            with (
                tc.tile_pool(name="wfc", bufs=2) as wfcp,
                tc.tile_pool(name="fc_ps", bufs=4, space="PSUM") as fc_ps,
            ):
                for oq in range(4):
                    wq = wfcp.tile([P, 8, 1024], dt.bfloat16, tag="wfcq")
                    nc.sync.dma_start(
                        wq[:],
                        w_fc[:, oq * 1024:(oq + 1) * 1024].rearrange(
                            "(c p) f -> p c f", p=P
                        ),
                    )
                    for oc8 in range(8):
                        oc = oq * 8 + oc8
                        ps = fc_ps.tile([P, 512], dt.float32, tag="fcps")
                        for c8 in range(8):
                            nc.tensor.matmul(
                                ps[:],
                                lhsT=wq[:, c8, oc8 * P:(oc8 + 1) * P],
                                rhs=n_bf[c8][:],
                                start=(c8 == 0),
                                stop=(c8 == 7),
                            )
                        nc.scalar.activation(
                            h1g[:, oc, :], ps[:], AF.Gelu,
                            bias=bfc_sb[:, oc:oc + 1],
                        )

            with tc.tile_pool(name="pr_ps", bufs=1, space="PSUM") as pr_ps:
                mps = [pr_ps.tile([P, 512], dt.float32, tag=f"mps{i}", name=f"mps{i}") for i in range(8)]
                for q4 in range(4):
                    wq = wpr_q0 if q4 == 0 else load_wpr(q4)
                    for oc in range(8):
                        for c8 in range(8):
                            nc.tensor.matmul(
                                mps[oc][:],
                                lhsT=wq[:, c8, oc * P:(oc + 1) * P],
                                rhs=h1g[:, q4 * 8 + c8, :],
                                start=(q4 == 0 and c8 == 0),
                                stop=(q4 == 3 and c8 == 7),
                            )
                for oc in range(8):
                    nc.vector.scalar_tensor_tensor(
                        r2[:, oc, :], mps[oc][:], bpr_sb[:, oc:oc + 1], n_sb[:, oc, :],
                        op0=OP.add, op1=OP.add,
                    )
            wprp_cm.__exit__(None, None, None)

            layernorm(r2, r2, g2_sb, b2_sb, "ln2")
            out_v = out_d.rearrange("(c p) t -> p c t", p=P)
            for c8 in range(8):
                nc.sync.dma_start(out_v[:, c8, :], r2[:, c8, :])
            pN_cm.__exit__(None, None, None)
            pR_cm.__exit__(None, None, None)

    nc.compile()
    return nc


def _prep_shared(w_attn, b_attn, w_o, b_o, ln1_g, ln1_b, w_fc, b_fc, w_pr, b_pr,
                 ln2_g, ln2_b):
    w_attn = np.asarray(w_attn, np.float32)
    b_attn = np.asarray(b_attn, np.float32)
    w_o_f = np.asarray(w_o, np.float32)
    b_v = b_attn[2 * D:]
    b_o_eff = (np.asarray(b_o, np.float32) + b_v @ w_o_f).astype(np.float32)
    mask = np.where(
        np.arange(P)[:, None] > np.arange(P)[None, :], MASK_NEG, 0.0
    ).astype(BF16)  # [ki, qj]: mask keys above the diagonal
    shared = {
        "w_qk": w_attn[:, : 2 * D].astype(BF16),
        "w_v": np.ascontiguousarray(w_attn[:, 2 * D:]).astype(BF16),
        "w_o": w_o_f.astype(BF16),
        "w_fc": np.asarray(w_fc, np.float32).astype(BF16),
        "w_pr": np.asarray(w_pr, np.float32).astype(BF16),
        "b_qk": np.ascontiguousarray(b_attn[: 2 * D]),
        "b_o": b_o_eff,
        "b_fc": np.asarray(b_fc, np.float32),
        "b_pr": np.asarray(b_pr, np.float32),
        "g1": np.asarray(ln1_g, np.float32),
        "b1": np.asarray(ln1_b, np.float32),
        "g2": np.asarray(ln2_g, np.float32),
        "b2": np.asarray(ln2_b, np.float32),
        "ident": np.eye(P, dtype=np.float32).astype(BF16),
        "maskm": mask,
    }
    return shared


def kernel(x, w_attn, b_attn, w_o, b_o, ln1_g, ln1_b, w_fc, b_fc, w_pr, b_pr,
           ln2_g, ln2_b, _trace=False, _tmpdir=None):
    from concourse.bass_utils import run_bass_kernel_spmd

    if "nc" not in _CACHE:
        _CACHE["nc"] = _build()
    nc = _CACHE["nc"]

    x = np.asarray(x, np.float32)
    shared = _prep_shared(w_attn, b_attn, w_o, b_o, ln1_g, ln1_b, w_fc, b_fc,
                          w_pr, b_pr, ln2_g, ln2_b)

    in_maps = []
    idxs = []
    for c in range(N_CORES):
        b, j = c // 4, c % 4
        idx = np.r_[j * 256:(j + 1) * 256, (7 - j) * 256:(8 - j) * 256]
        idxs.append((b, idx))
        xT = np.ascontiguousarray(x[b, idx, :].T)
        m = dict(shared)
        m["xT_f32"] = xT
        m["xT_bf"] = xT.astype(BF16)
        m["jidx"] = np.array([[j]], np.uint32)
        in_maps.append(m)

    res = run_bass_kernel_spmd(
        nc, in_maps, core_ids=list(range(N_CORES)), trace=_trace, tmpdir=_tmpdir
    )
    if _trace:
        _CACHE["exec_time_ns"] = res.exec_time_ns

    out = np.empty((B, S, D), np.float32)
    for c in range(N_CORES):
        b, idx = idxs[c]
        out[b, idx, :] = res.results[c]["out"].T
    return out



# revision 16
# speedup vs baseline: 1.3361x; 1.3361x over previous
"""Dense transformer block (attention + post-LN MLP) on 8 trn2 NeuronCores.

Context-parallel sharding: core c handles batch c//4 and token blocks
j*256, (7-j)*256 of that batch (j = c%4) -> causally balanced 512
tokens/core. Weights replicated. fp8e4 DoubleRow for qkv/w_o/w_pr
projections (w_fc stays bf16 for precision), K/V/Q/softmax weights fp8
(halves the K/V AllGather), residual stream bf16. AG order k0,v0,v1,k1
queued to start right at the CC barrier. Attention emission is
software-pipelined (AV deferred one chunk) so the in-order PE queue
never stalls behind exp. Activations feature-major (x^T) end-to-end.
"""

import numpy as np
import ml_dtypes

BF16 = ml_dtypes.bfloat16
FP8 = ml_dtypes.float8_e4m3  # TRN float8e4 (E4M3, max +-240)

N_CORES = 8
B, S, D = 2, 2048, 1024
H, HD = 16, 64
F = 4 * D
TOK = 512            # tokens per core
P = 128
NHP = H // 2         # 8 head pairs
MASK_NEG = -80000.0  # /8 => exp underflows to exactly 0
LN_EPS = 1e-5
WSC = 64.0           # fp8 scale for DR weights (std 0.02 -> 1.28)

_CACHE = {}


def _q_blocks(j):
    """Global 128-blocks of this core's query tokens, ascending."""
    return [2 * j, 2 * j + 1, 14 - 2 * j, 15 - 2 * j]


def _build():
    import concourse.bass as bass
    import concourse.mybir as mybir
    import concourse.tile as tile
    from concourse import bacc

    dt = mybir.dt
    AF = mybir.ActivationFunctionType
    OP = mybir.AluOpType
    DR = mybir.MatmulPerfMode.DoubleRow

    nc = bacc.Bacc(
        "TRN2",
        target_bir_lowering=False,
        debug=False,
        enable_asserts=True,
        num_devices=N_CORES,
    )

    def din(name, shape, dty):
        return nc.dram_tensor(name, shape, dty, kind="ExternalInput").ap()

    # all host-side pre-arranged: partition-major, fully contiguous DMA
    xT_bf = din("xT_bf", [P, 8, TOK], dt.bfloat16)
    x8_d = din("x8", [P, 4, 2, TOK], dt.float8e4)
    w_qk = din("w_qk", [4, P, 4, 2, 512], dt.float8e4)    # *WSC
    w_v = din("w_v", [P, 4, 2, D], dt.float8e4)           # *WSC
    w_o = din("w_o", [P, 4, 2, D], dt.float8e4)           # *WSC
    w_fc = din("w_fc", [4, P, 8, 1024], dt.bfloat16)
    w_pr = din("w_pr", [4, P, 4, 2, 1024], dt.float8e4)   # *WSC
    b_qk = din("b_qk", [P, 16], dt.float32)
    b_o = din("b_o", [P, 8], dt.float32)
    b_fc = din("b_fc", [P, 32], dt.float32)
    b_pr = din("b_pr", [P, 8], dt.float32)
    g1 = din("g1", [P, 8], dt.float32)
    b1 = din("b1", [P, 8], dt.float32)
    g2 = din("g2", [P, 8], dt.float32)
    b2 = din("b2", [P, 8], dt.float32)
    ident_d = din("ident", [P, P], dt.bfloat16)
    maskm_d = din("maskm", [P, P], dt.bfloat16)
    jidx_d = din("jidx", [1, 1], dt.uint32)
    out_d = nc.dram_tensor("out", [P, 8, TOK], dt.float32, kind="ExternalOutput").ap()

    VW = 66  # per-head V row width: 64 v + ones col (64) + pad (65)

    with tile.TileContext(nc) as tc:
        from contextlib import ExitStack

        ctx = ExitStack()
        with ctx:
            c_pool = ctx.enter_context(tc.tile_pool(name="consts", bufs=1))
            dram = ctx.enter_context(tc.tile_pool(name="dram", bufs=1, space="DRAM"))

            # ---- per-core j register (for Switch) ----
            jreg = nc.alloc_registers(
                "jreg",
                [mybir.EngineType.PE, mybir.EngineType.Activation,
                 mybir.EngineType.DVE, mybir.EngineType.SP],
            )
            nc.regs_load(jreg, jidx_d[0:1, 0:1])
            jval = nc.snap(jreg, donate=True, min_val=0, max_val=3)

            # ---- constants ----
            ident = c_pool.tile([P, P], dt.bfloat16, tag="ident")
            nc.sync.dma_start(ident[:], ident_d[:])
            maskm = c_pool.tile([P, P], dt.bfloat16, tag="maskm")
            nc.sync.dma_start(maskm[:], maskm_d[:])
            ones128_bf = c_pool.tile([P, 1], dt.bfloat16, tag="ones128")
            nc.vector.memset(ones128_bf[:], 1.0)
            ones1_bf = c_pool.tile([1, P], dt.bfloat16, tag="ones1b")
            nc.vector.memset(ones1_bf[:], 1.0)
            ones_hi = c_pool.tile([65, 64], dt.bfloat16, tag="oneshi")
            nc.vector.memset(ones_hi[64:65, :], 1.0)

            bqk_sb = c_pool.tile([P, 16], dt.float32, tag="bqk")
            nc.sync.dma_start(bqk_sb[:], b_qk[:])
            bo_sb = c_pool.tile([P, 8], dt.float32, tag="bo")
            nc.sync.dma_start(bo_sb[:], b_o[:])
            bfc_sb = c_pool.tile([P, 32], dt.float32, tag="bfc")
            nc.sync.dma_start(bfc_sb[:], b_fc[:])
            bpr_sb = c_pool.tile([P, 8], dt.float32, tag="bpr")
            nc.sync.dma_start(bpr_sb[:], b_pr[:])
            g1_sb = c_pool.tile([P, 8], dt.float32, tag="g1")
            nc.sync.dma_start(g1_sb[:], g1[:])
            b1_sb = c_pool.tile([P, 8], dt.float32, tag="b1")
            nc.sync.dma_start(b1_sb[:], b1[:])
            g2_sb = c_pool.tile([P, 8], dt.float32, tag="g2")
            nc.sync.dma_start(g2_sb[:], g2[:])
            b2_sb = c_pool.tile([P, 8], dt.float32, tag="b2")
            nc.sync.dma_start(b2_sb[:], b2[:])

            # ---- AG buffers: 4 fp8 collectives, order k0,v0,v1,k1 ----
            KH = 4 * P * TOK                 # half of kT  [4,128,512]
            VH = 2 * P * (16 * VW)           # half of v   [2,128,1056]
            ag_k_in = [dram.tile([KH], dt.float8e4, tag=f"agki{i}", name=f"agki{i}") for i in range(2)]
            ag_k_out = [dram.tile([4, KH], dt.float8e4, tag=f"agko{i}", name=f"agko{i}") for i in range(2)]
            ag_v_in = [dram.tile([VH], dt.float8e4, tag=f"agvi{i}", name=f"agvi{i}") for i in range(2)]
            ag_v_out = [dram.tile([4, VH], dt.float8e4, tag=f"agvo{i}", name=f"agvo{i}") for i in range(2)]
            GROUPS = [[0, 1, 2, 3], [4, 5, 6, 7]]

            # ---- residual/MLP tiles live from phase 4 to the end ----
            pN_cm = tc.tile_pool(name="pN", bufs=1)
            pN = pN_cm.__enter__()
            r1_bf = pN.tile([P, 8, TOK], dt.bfloat16, tag="r1bf")
            n_bf = pN.tile([P, 8, TOK], dt.bfloat16, tag="n_bf")
            h1g = pN.tile([P, 32, TOK], dt.float8e4, tag="h1g")
            r2_bf = pN.tile([P, 8, TOK], dt.bfloat16, tag="r2bf")
            wfc0 = pN.tile([P, 8, 1024], dt.bfloat16, tag="wfc0")

            # ---- qT / o_cat / wo live from phase 1 through w_o ----
            pA_cm = tc.tile_pool(name="pA", bufs=1)
            pA = pA_cm.__enter__()
            qT = pA.tile([P, 8, TOK], dt.float8e4, tag="qT")
            o_cat = pA.tile([P, 8, TOK], dt.float8e4, tag="o_cat")
            o_tmp = [pA.tile([64, TOK], dt.float8e4, tag=f"o_tmp{i}", name=f"o_tmp{i}") for i in range(NHP)]
            wo8 = pA.tile([P, 4, 2, D], dt.float8e4, tag="wo8")
            xfs_bf = pA.tile([P, 8, TOK], dt.bfloat16, tag="xfs")

            # gathered K/V + own-kv pools open across phase1+attention
            gat_cm = tc.tile_pool(name="gathered", bufs=1)
            gat = gat_cm.__enter__()
            kT_pack = gat.tile([P, 8, S], dt.float8e4, tag="kT_pack")
            v_ones = gat.tile([P, 16, 16 * VW], dt.float8e4, tag="v_ones")
            kvp_cm = tc.tile_pool(name="kv_own", bufs=1)
            kvp = kvp_cm.__enter__()
            v_own = kvp.tile([P, 4, 16 * VW], dt.float8e4, tag="v_own")
            kT_own = kvp.tile([P, 8, TOK], dt.float8e4, tag="kT_own")

            # ============ phase 1: qkv (fp8 DoubleRow) ============
            with (
                tc.tile_pool(name="xw", bufs=2) as xw,
                tc.tile_pool(name="qkv_ps", bufs=4, space="PSUM") as qkv_ps,
            ):
                xb8 = xw.tile([P, 4, 2, TOK], dt.float8e4, tag="xb8")
                nc.sync.dma_start(xb8[:], x8_d[:])
                wq2 = xw.tile([P, 4, 2, 512], dt.float8e4, tag="wqkq", name="wq2")
                nc.sync.dma_start(wq2[:], w_qk[2])
                wv = xw.tile([P, 4, 2, D], dt.float8e4, tag="wv")
                nc.sync.dma_start(wv[:], w_v[:])

                v4 = v_own.rearrange("p c (h w) -> p c h w", w=VW)
                nc.vector.memset(v4[:, :, :, 64:66], 0.0)
                nc.vector.memset(v4[:, :, :, 64:65], 1.0)

                def v_block(t4):
                    for vc in range(2):
                        ps = qkv_ps.tile([P, 512], dt.float32, tag="qkvps")
                        for kc2 in range(4):
                            nc.tensor.matmul(
                                ps[:],
                                lhsT=xb8[:, kc2, :, t4 * P:(t4 + 1) * P],
                                rhs=wv[:, kc2, :, vc * 512:(vc + 1) * 512],
                                start=(kc2 == 0),
                                stop=(kc2 == 3),
                                perf_mode=DR,
                            )
                        dest = v4[:, t4, vc * 8:(vc + 1) * 8, 0:64]
                        nc.vector.tensor_scalar(
                            dest, ps[:].rearrange("p (h w) -> p h w", w=64),
                            1.0 / WSC, None, OP.mult,
                        )

                def ship_v(i):
                    nc.sync.dma_start(
                        ag_v_in[i].rearrange("(c p f) -> p c f", c=2, p=P),
                        v_own[:, 2 * i:2 * i + 2, :],
                    )
                    nc.gpsimd.collective_compute(
                        "AllGather", mybir.AluOpType.bypass,
                        replica_groups=GROUPS,
                        ins=[ag_v_in[i].opt()], outs=[ag_v_out[i].opt()],
                    )

                def ship_k(i):
                    nc.sync.dma_start(
                        ag_k_in[i].rearrange("(c p t) -> p c t", c=4, p=P),
                        kT_own[:, 4 * i:4 * i + 4, :],
                    )
                    nc.gpsimd.collective_compute(
                        "AllGather", mybir.AluOpType.bypass,
                        replica_groups=GROUPS,
                        ins=[ag_k_in[i].opt()], outs=[ag_k_out[i].opt()],
                    )

                def qk_quarter(qi, wq):
                    # w_qk columns: [q(0:1024) | k(1024:2048)]
                    for fo in range(4):
                        fchunk = 4 * qi + fo
                        ps = qkv_ps.tile([P, 512], dt.float32, tag="qkvps")
                        for kc2 in range(4):
                            nc.tensor.matmul(
                                ps[:],
                                lhsT=wq[:, kc2, :, fo * P:(fo + 1) * P],
                                rhs=xb8[:, kc2, :, :],
                                start=(kc2 == 0),
                                stop=(kc2 == 3),
                                perf_mode=DR,
                            )
                        dest = kT_own[:, fchunk - 8, :] if fchunk >= 8 else qT[:, fchunk, :]
                        nc.scalar.activation(
                            dest, ps[:], AF.Identity,
                            bias=bqk_sb[:, fchunk:fchunk + 1], scale=1.0 / WSC,
                        )

                # order: kq2 -> AGk0; v01 -> AGv0; v23 -> AGv1; kq3 -> AGk1;
                # then q quarters (only needed by local attention pass 1).
                qk_quarter(2, wq2)
                ship_k(0)
                v_block(0)
                v_block(1)
                ship_v(0)
                v_block(2)
                v_block(3)
                ship_v(1)
                wq3 = xw.tile([P, 4, 2, 512], dt.float8e4, tag="wqkq", name="wq3")
                nc.sync.dma_start(wq3[:], w_qk[3])
                qk_quarter(3, wq3)
                ship_k(1)
                wq0 = xw.tile([P, 4, 2, 512], dt.float8e4, tag="wqkq", name="wq0")
                nc.sync.dma_start(wq0[:], w_qk[0])
                qk_quarter(0, wq0)
                wq1 = xw.tile([P, 4, 2, 512], dt.float8e4, tag="wqkq", name="wq1")
                nc.sync.dma_start(wq1[:], w_qk[1])
                qk_quarter(1, wq1)

            # prefetch (DMA engines, overlaps attention)
            nc.sync.dma_start(wo8[:], w_o[:])
            nc.sync.dma_start(xfs_bf[:], xT_bf[:])
            nc.sync.dma_start(wfc0[:], w_fc[0])

            # ============ phase 2+3: K/V load + attention ============
            with (
                tc.tile_pool(name="att_s", bufs=2, space="PSUM") as s_ps_pool,
                tc.tile_pool(name="att_o", bufs=1, space="PSUM") as o_ps_pool,
                tc.tile_pool(name="att_rep", bufs=1, space="PSUM") as rep_ps_pool,
                tc.tile_pool(name="att_tmp", bufs=3) as atmp,
                tc.tile_pool(name="att_tmp2", bufs=2) as atmp2,
                tc.tile_pool(name="att_loc", bufs=1) as alocp,
            ):
                def _load_k(r, i, j):
                    if r == j:
                        kk = kT_own[:, 4 * i:4 * i + 4, :]
                    else:
                        kk = ag_k_out[i][r].rearrange("(c p t) -> p c t", c=4, p=P)
                    for hc in range(4):
                        hp = 4 * i + hc
                        nc.sync.dma_start(
                            kT_pack[:, hp, r * 256:(r + 1) * 256], kk[:, hc, 0:256])
                        nc.sync.dma_start(
                            kT_pack[:, hp, (7 - r) * 256:(8 - r) * 256],
                            kk[:, hc, 256:512])

                def _load_v(r, i, j):
                    if r == j:
                        vv = v_own[:, 2 * i:2 * i + 2, :]
                    else:
                        vv = ag_v_out[i][r].rearrange("(c p f) -> p c f", c=2, p=P)
                    for lc in range(2):
                        lt = 2 * i + lc
                        gb = r if lt < 2 else 7 - r
                        g16 = 2 * gb + (lt % 2)
                        nc.sync.dma_start(v_ones[:, g16, :], vv[:, lc, :])

                def load_kv(j):
                    """Fill kT_pack / v_ones.  Own rank straight from SBUF;
                    remote halves in AG completion order (k0, v0, v1, k1)."""
                    for i in range(2):
                        _load_k(j, i, j)
                        _load_v(j, i, j)
                    rem = [r for r in range(4) if r != j]
                    for r in rem:
                        _load_k(r, 0, j)
                    for r in rem:
                        _load_v(r, 0, j)
                    for r in rem:
                        _load_v(r, 1, j)
                    for r in rem:
                        _load_k(r, 1, j)

                # ====== attention: QK+exp now, AV deferred one chunk so the
                # in-order PE queue never stalls on exp ======
                def attn_qk_exp(hp, kc, gq):
                    vi = [i for i in range(4) if gq[i] >= kc]
                    qs = P * vi[0]
                    diag = gq[vi[0]] == kc
                    sps = s_ps_pool.tile([P, 2, TOK], dt.float32, tag="sps")
                    for h2 in range(2):
                        lo, hi = 64 * h2, 64 * (h2 + 1)
                        nc.tensor.matmul(
                            sps[:, h2, qs:],
                            lhsT=kT_pack[lo:hi, hp, kc * P:(kc + 1) * P],
                            rhs=qT[lo:hi, hp, qs:],
                            start=True,
                            stop=not diag,
                            tile_position=(64 * h2, 0),
                        )
                        if diag:
                            nc.tensor.matmul(
                                sps[:, h2, qs:qs + P],
                                lhsT=ident[:],
                                rhs=maskm[:],
                                start=False,
                                stop=True,
                            )
                    aT = atmp.tile([P, 2, TOK], dt.float8e4, tag="aT")
                    nc.scalar.activation(
                        aT[:, :, qs:], sps[:, :, qs:], AF.Exp, scale=0.125
                    )
                    return (hp, kc, qs, aT)

                def attn_av(oT, pend, first, last):
                    hp, kc, qs, aT = pend
                    for h2 in range(2):
                        h = 2 * hp + h2
                        nc.tensor.matmul(
                            oT[:, h2, qs:],
                            lhsT=v_ones[:, kc, h * VW:h * VW + 65],
                            rhs=aT[:, h2, qs:],
                            start=first,
                            stop=last,
                        )

                def attn_chunks(hp, chunks, gq, oT):
                    pend = None
                    for ki, kc in enumerate(chunks):
                        cur = attn_qk_exp(hp, kc, gq)
                        if pend is not None:
                            attn_av(oT, pend, ki == 1, False)
                        pend = cur
                    attn_av(oT, pend, len(chunks) == 1, True)

                def attn_arm(j):
                    load_kv(j)
                    gq = _q_blocks(j)
                    last_kc = gq[-1]
                    remote = [k for k in range(last_kc + 1) if k not in gq]
                    # pass 1: all local chunks (overlap the AllGather chain)
                    oloc = []
                    for hp in range(NHP):
                        oT = o_ps_pool.tile([65, 2, TOK], dt.float32, tag="oT")
                        attn_chunks(hp, gq, gq, oT)
                        ol = alocp.tile([65, 2, TOK], dt.float32,
                                       tag=f"oloc{hp}", name=f"ol{hp}")
                        nc.vector.tensor_copy(ol[:], oT[:])
                        oloc.append(ol)
                    # pass 2: remote chunks + local partial + normalize
                    qs0 = P * [i for i in range(4) if gq[i] >= remote[0]][0]
                    for hp in range(NHP):
                        oT = o_ps_pool.tile([65, 2, TOK], dt.float32, tag="oT")
                        attn_chunks(hp, remote, gq, oT)
                        osum = atmp2.tile([65, 2, TOK], dt.float32, tag="osum")
                        if qs0 > 0:
                            nc.vector.tensor_copy(osum[:, :, 0:qs0], oloc[hp][:, :, 0:qs0])
                        nc.vector.tensor_tensor(
                            osum[:, :, qs0:], oT[:, :, qs0:], oloc[hp][:, :, qs0:], OP.add)
                        # denominator row -> bf16 -> PE broadcast -> DVE recip
                        rrow = atmp2.tile([65, 2, TOK], dt.bfloat16, tag="rrow")
                        nc.vector.tensor_copy(rrow[64:65, :, :], osum[64:65, :, :])
                        rep = rep_ps_pool.tile([64, 2, TOK], dt.float32, tag="rep")
                        for h2 in range(2):
                            nc.tensor.matmul(
                                rep[:, h2, :],
                                lhsT=ones_hi[64:65, :],
                                rhs=rrow[64:65, h2, :],
                                start=True,
                                stop=True,
                            )
                        rrec = atmp2.tile([64, 2, TOK], dt.float32, tag="rrec")
                        nc.vector.reciprocal_approx_fast(out=rrec[:], in_=rep[:])
                        for h2 in range(2):
                            dest = o_cat[0:64, hp, :] if h2 == 0 else o_tmp[hp][:]
                            nc.vector.tensor_tensor(
                                dest, osum[0:64, h2, :], rrec[:, h2, :], OP.mult
                            )
                        nc.sync.dma_start(o_cat[64:128, hp, :], o_tmp[hp][:])

                for case in tc.Switch(jval, 4):
                    attn_arm(case)

            kvp_cm.__exit__(None, None, None)
            gat_cm.__exit__(None, None, None)

            # ============ layernorm finalize helper (stats already in psum) ====
            def ln_finalize(lnp, lnrep, s1, s2, tag):
                mu = lnp.tile([1, TOK], dt.float32, tag=f"mu_{tag}")
                nc.vector.tensor_scalar(mu[:], s1[:], 1.0 / D, None, OP.mult)
                m2 = lnp.tile([1, TOK], dt.float32, tag=f"m2_{tag}")
                nc.vector.tensor_scalar(m2[:], s2[:], 1.0 / D, LN_EPS, OP.mult, OP.add)
                var = lnp.tile([1, TOK], dt.float32, tag=f"var_{tag}")
                nc.vector.tensor_tensor(var[:], mu[:], mu[:], OP.mult)
                nc.vector.tensor_tensor(var[:], m2[:], var[:], OP.subtract)
                lnv = lnp.tile([1, TOK], dt.float32, tag=f"lnv_{tag}")
                nc.scalar.activation(lnv[:], var[:], AF.Ln)
                rsmu = lnp.tile([1, 2, TOK], dt.bfloat16, tag=f"rsmu_{tag}")
                nc.scalar.activation(rsmu[:, 0, :], lnv[:], AF.Exp, scale=-0.5)
                nc.vector.tensor_tensor(
                    rsmu[:, 1, :], mu[:], rsmu[:, 0, :], OP.mult
                )
                rep = lnrep.tile([P, 2, TOK], dt.float32, tag=f"rep_{tag}")
                for i in range(2):
                    nc.tensor.matmul(
                        rep[:, i, :], lhsT=ones1_bf[:], rhs=rsmu[:, i, :],
                        start=True, stop=True,
                    )
                return rep

            # ============ phase 4: w_o (fp8 DR) + residual + LN1 ============
            with (
                tc.tile_pool(name="wo_ps", bufs=4, space="PSUM") as wo_ps,
                tc.tile_pool(name="ln1_s", bufs=1, space="PSUM") as ln1_s,
                tc.tile_pool(name="ln1_t", bufs=2) as ln1_t,
                tc.tile_pool(name="ln1_f", bufs=1) as ln1_f,
                tc.tile_pool(name="ln1_r", bufs=1, space="PSUM") as ln1_r,
            ):
                s1 = ln1_s.tile([1, TOK], dt.float32, tag="s1")
                s2 = ln1_s.tile([1, TOK], dt.float32, tag="s2")
                for oc in range(8):
                    ps = wo_ps.tile([P, 512], dt.float32, tag="wops")
                    for kc2 in range(4):
                        nc.tensor.matmul(
                            ps[:],
                            lhsT=wo8[:, kc2, :, oc * P:(oc + 1) * P],
                            rhs=o_cat[:, 2 * kc2:2 * kc2 + 2, :],
                            start=(kc2 == 0),
                            stop=(kc2 == 3),
                            perf_mode=DR,
                        )
                    t0 = ln1_t.tile([P, TOK], dt.float32, tag="t0")
                    nc.vector.tensor_scalar(
                        t0[:], ps[:], 1.0 / WSC, bo_sb[:, oc:oc + 1], OP.mult, OP.add
                    )
                    nc.vector.tensor_tensor(r1_bf[:, oc, :], t0[:], xfs_bf[:, oc, :], OP.add)
                    sq = ln1_t.tile([P, TOK], dt.bfloat16, tag="sq1")
                    nc.vector.tensor_tensor(sq[:], r1_bf[:, oc, :], r1_bf[:, oc, :], OP.mult)
                    nc.tensor.matmul(
                        s1[:], lhsT=ones128_bf[:], rhs=r1_bf[:, oc, :],
                        start=(oc == 0), stop=(oc == 7),
                    )
                    nc.tensor.matmul(
                        s2[:], lhsT=ones128_bf[:], rhs=sq[:],
                        start=(oc == 0), stop=(oc == 7),
                    )
                rep1 = ln_finalize(ln1_f, ln1_r, s1, s2, "ln1")
                for c8 in range(8):
                    t = ln1_t.tile([P, TOK], dt.float32, tag="t1")
                    nc.vector.tensor_tensor(
                        t[:], r1_bf[:, c8, :], rep1[:, 0, :], OP.mult
                    )
                    nc.vector.tensor_tensor(t[:], t[:], rep1[:, 1, :], OP.subtract)
                    nc.vector.tensor_scalar(
                        n_bf[:, c8, :], t[:], g1_sb[:, c8:c8 + 1], b1_sb[:, c8:c8 + 1],
                        OP.mult, OP.add,
                    )

            pA_cm.__exit__(None, None, None)

            # ============ phase 5: MLP (fc bf16, pr fp8 DR) ============
            ln2_s_cm = tc.tile_pool(name="ln2_s", bufs=1, space="PSUM")
            ln2_s = ln2_s_cm.__enter__()
            s1b = ln2_s.tile([1, TOK], dt.float32, tag="s1b")
            s2b = ln2_s.tile([1, TOK], dt.float32, tag="s2b")
            wprp_cm = tc.tile_pool(name="wpr", bufs=1)
            wprp = wprp_cm.__enter__()
            wpr_sb = wprp.tile([P, 4, 4, 2, 1024], dt.float8e4, tag="wprq")
            nc.sync.dma_start(wpr_sb[:, 0], w_pr[0])

            with (
                tc.tile_pool(name="wfc", bufs=2) as wfcp,
                tc.tile_pool(name="fc_ps", bufs=4, space="PSUM") as fc_ps,
            ):
                for oq in range(4):
                    if oq == 0:
                        wq = wfc0
                    else:
                        wq = wfcp.tile([P, 8, 1024], dt.bfloat16, tag="wfcq")
                        nc.sync.dma_start(wq[:], w_fc[oq])
                        nc.sync.dma_start(wpr_sb[:, oq], w_pr[oq])
                    for oc8 in range(8):
                        oc = oq * 8 + oc8
                        ps = fc_ps.tile([P, 512], dt.float32, tag="fcps")
                        for c8 in range(8):
                            nc.tensor.matmul(
                                ps[:],
                                lhsT=wq[:, c8, oc8 * P:(oc8 + 1) * P],
                                rhs=n_bf[:, c8, :],
                                start=(c8 == 0),
                                stop=(c8 == 7),
                            )
                        nc.scalar.activation(
                            h1g[:, oc, :], ps[:], AF.Gelu,
                            bias=bfc_sb[:, oc:oc + 1],
                        )

            with (
                tc.tile_pool(name="pr_ps", bufs=2, space="PSUM") as pr_ps,
                tc.tile_pool(name="ln2_t", bufs=2) as ln2_t,
            ):
                for oc in range(8):
                    ps = pr_ps.tile([P, 512], dt.float32, tag="prps")
                    for q4 in range(4):
                        for kc2 in range(4):
                            nc.tensor.matmul(
                                ps[:],
                                lhsT=wpr_sb[:, q4, kc2, :, oc * P:(oc + 1) * P],
                                rhs=h1g[:, 8 * q4 + 2 * kc2:8 * q4 + 2 * kc2 + 2, :],
                                start=(q4 == 0 and kc2 == 0),
                                stop=(q4 == 3 and kc2 == 3),
                                perf_mode=DR,
                            )
                    t = ln2_t.tile([P, TOK], dt.float32, tag="tm")
                    nc.vector.tensor_scalar(
                        t[:], ps[:], 1.0 / WSC, bpr_sb[:, oc:oc + 1], OP.mult, OP.add
                    )
                    nc.vector.tensor_tensor(r2_bf[:, oc, :], t[:], n_bf[:, oc, :], OP.add)
                    sq = ln2_t.tile([P, TOK], dt.bfloat16, tag="sq2")
                    nc.vector.tensor_tensor(sq[:], r2_bf[:, oc, :], r2_bf[:, oc, :], OP.mult)
                    nc.tensor.matmul(
                        s1b[:], lhsT=ones128_bf[:], rhs=r2_bf[:, oc, :],
                        start=(oc == 0), stop=(oc == 7),
                    )
                    nc.tensor.matmul(
                        s2b[:], lhsT=ones128_bf[:], rhs=sq[:],
                        start=(oc == 0), stop=(oc == 7),
                    )
            wprp_cm.__exit__(None, None, None)

            with (
                tc.tile_pool(name="ln2_f", bufs=1) as ln2_f,
                tc.tile_pool(name="ln2_r", bufs=1, space="PSUM") as ln2_r,
                tc.tile_pool(name="ln2_o", bufs=2) as ln2_o,
            ):
                rep2 = ln_finalize(ln2_f, ln2_r, s1b, s2b, "ln2")
                for c8 in range(8):
                    t = ln2_o.tile([P, TOK], dt.float32, tag="t2")
                    nc.vector.tensor_tensor(
                        t[:], r2_bf[:, c8, :], rep2[:, 0, :], OP.mult
                    )
                    nc.vector.tensor_tensor(t[:], t[:], rep2[:, 1, :], OP.subtract)
                    h = ln2_o.tile([P, TOK], dt.float32, tag="h2o")
                    nc.vector.tensor_scalar(
                        h[:], t[:], g2_sb[:, c8:c8 + 1], b2_sb[:, c8:c8 + 1],
                        OP.mult, OP.add,
                    )
                    nc.sync.dma_start(out_d[:, c8, :], h[:])
            ln2_s_cm.__exit__(None, None, None)
            pN_cm.__exit__(None, None, None)

    nc.compile()
    return nc


def _prep_shared(w_attn, b_attn, w_o, b_o, ln1_g, ln1_b, w_fc, b_fc, w_pr, b_pr,
                 ln2_g, ln2_b):
    w_attn = np.asarray(w_attn, np.float32)
    b_attn = np.asarray(b_attn, np.float32)
    w_o_f = np.asarray(w_o, np.float32)
    b_v = b_attn[2 * D:]
    b_o_eff = (np.asarray(b_o, np.float32) + b_v @ w_o_f).astype(np.float32)
    mask = np.where(
        np.arange(P)[:, None] > np.arange(P)[None, :], MASK_NEG, 0.0
    ).astype(BF16)  # [ki, qj]: mask keys above the diagonal

    def pc8(w):  # [D, X] -> [128, 8, X] partition-major
        return np.ascontiguousarray(
            w.reshape(8, P, -1).transpose(1, 0, 2))

    def pdr(w):  # [1024, X] (contraction) -> [128, 4, 2, X] DoubleRow fp8
        return np.ascontiguousarray(
            (w * WSC).reshape(4, 2, P, -1).transpose(2, 0, 1, 3)).astype(FP8)

    def col(b, n):  # [n*128] -> [128, n]
        return np.ascontiguousarray(b.reshape(n, P).T)

    w_qk_q = np.stack([pdr(w_attn[:, 512 * qi:512 * (qi + 1)]) for qi in range(4)])
    wfc = np.asarray(w_fc, np.float32)
    wpr = np.asarray(w_pr, np.float32)
    w_fc_q = np.stack([pc8(wfc[:, 1024 * oq:1024 * (oq + 1)]).astype(BF16)
                       for oq in range(4)])
    w_pr_q = np.stack([pdr(wpr[1024 * q4:1024 * (q4 + 1), :]) for q4 in range(4)])

    shared = {
        "w_qk": w_qk_q,
        "w_v": pdr(w_attn[:, 2 * D:]),
        "w_o": pdr(w_o_f),
        "w_fc": w_fc_q,
        "w_pr": w_pr_q,
        "b_qk": col(b_attn[: 2 * D], 16),
        "b_o": col(b_o_eff, 8),
        "b_fc": col(np.asarray(b_fc, np.float32), 32),
        "b_pr": col(np.asarray(b_pr, np.float32), 8),
        "g1": col(np.asarray(ln1_g, np.float32), 8),
        "b1": col(np.asarray(ln1_b, np.float32), 8),
        "g2": col(np.asarray(ln2_g, np.float32), 8),
        "b2": col(np.asarray(ln2_b, np.float32), 8),
        "ident": np.eye(P, dtype=np.float32).astype(BF16),
        "maskm": mask,
    }
    return shared


def kernel(x, w_attn, b_attn, w_o, b_o, ln1_g, ln1_b, w_fc, b_fc, w_pr, b_pr,
           ln2_g, ln2_b, _trace=False, _tmpdir=None):
    from concourse.bass_utils import run_bass_kernel_spmd

    if "nc" not in _CACHE:
        _CACHE["nc"] = _build()
    nc = _CACHE["nc"]

    x = np.asarray(x, np.float32)
    shared = _prep_shared(w_attn, b_attn, w_o, b_o, ln1_g, ln1_b, w_fc, b_fc,
                          w_pr, b_pr, ln2_g, ln2_b)

    in_maps = []
    idxs = []
    for c in range(N_CORES):
        b, j = c // 4, c % 4
        idx = np.r_[j * 256:(j + 1) * 256, (7 - j) * 256:(8 - j) * 256]
        idxs.append((b, idx))
        xT = x[b, idx, :].T  # [D, TOK]
        xb = np.ascontiguousarray(xT.reshape(8, P, TOK).transpose(1, 0, 2)).astype(BF16)
        x8 = np.ascontiguousarray(
            xb.astype(np.float32).reshape(P, 4, 2, TOK)).astype(FP8)
        m = dict(shared)
        m["xT_bf"] = xb
        m["x8"] = x8
        m["jidx"] = np.array([[j]], np.uint32)
        in_maps.append(m)

    res = run_bass_kernel_spmd(
        nc, in_maps, core_ids=list(range(N_CORES)), trace=_trace, tmpdir=_tmpdir
    )
    if _trace:
        _CACHE["exec_time_ns"] = res.exec_time_ns

    out = np.empty((B, S, D), np.float32)
    for c in range(N_CORES):
        b, idx = idxs[c]
        o = res.results[c]["out"]  # [128, 8, 512]
        out[b, idx, :] = o.transpose(1, 0, 2).reshape(D, TOK).T
    return out


# revision 23
# speedup vs baseline: 1.3787x; 1.0319x over previous
"""Dense transformer block (attention + post-LN MLP) on 8 trn2 NeuronCores.

Context-parallel sharding: core c handles batch c//4 and token blocks
j*256, (7-j)*256 of that batch (j = c%4) -> causally balanced 512
tokens/core. Weights replicated. fp8e4 DoubleRow for qkv/w_o/w_pr
projections (w_fc stays bf16 for precision), K/V/Q/softmax weights fp8
(halves the K/V AllGather), residual stream bf16. AG order k0,v0,v1,k1
queued to start right at the CC barrier. Attention emission is
software-pipelined (AV deferred one chunk) so the in-order PE queue
never stalls behind exp. Activations feature-major (x^T) end-to-end.
"""

import numpy as np
import ml_dtypes

BF16 = ml_dtypes.bfloat16
FP8 = ml_dtypes.float8_e4m3  # TRN float8e4 (E4M3, max +-240)

N_CORES = 8
B, S, D = 2, 2048, 1024
H, HD = 16, 64
F = 4 * D
TOK = 512            # tokens per core
P = 128
NHP = H // 2         # 8 head pairs
MASK_NEG = -80000.0  # /8 => exp underflows to exactly 0
LN_EPS = 1e-5
WSC = 64.0           # fp8 scale for DR weights (std 0.02 -> 1.28)

_CACHE = {}


def _q_blocks(j):
    """Global 128-blocks of this core's query tokens, ascending."""
    return [2 * j, 2 * j + 1, 14 - 2 * j, 15 - 2 * j]


def _build():
    import concourse.bass as bass
    import concourse.mybir as mybir
    import concourse.tile as tile
    from concourse import bacc

    dt = mybir.dt
    AF = mybir.ActivationFunctionType
    OP = mybir.AluOpType
    DR = mybir.MatmulPerfMode.DoubleRow

    nc = bacc.Bacc(
        "TRN2",
        target_bir_lowering=False,
        debug=False,
        enable_asserts=True,
        num_devices=N_CORES,
    )

    def din(name, shape, dty):
        return nc.dram_tensor(name, shape, dty, kind="ExternalInput").ap()

    # all host-side pre-arranged: partition-major, fully contiguous DMA
    xT_bf = din("xT_bf", [P, 8, TOK], dt.bfloat16)
    x8_d = din("x8", [P, 4, 2, TOK], dt.float8e4)
    w_qk = din("w_qk", [4, P, 4, 2, 512], dt.float8e4)    # *WSC
    w_v = din("w_v", [P, 4, 2, D], dt.float8e4)           # *WSC
    w_o = din("w_o", [P, 4, 2, D], dt.float8e4)           # *WSC
    w_fc = din("w_fc", [4, P, 8, 1024], dt.bfloat16)
    w_pr = din("w_pr", [4, P, 4, 2, 1024], dt.float8e4)   # *WSC
    b_qk = din("b_qk", [P, 16], dt.float32)
    b_o = din("b_o", [P, 8], dt.float32)
    b_fc = din("b_fc", [P, 32], dt.float32)
    b_pr = din("b_pr", [P, 8], dt.float32)
    g1 = din("g1", [P, 8], dt.float32)
    b1 = din("b1", [P, 8], dt.float32)
    g2 = din("g2", [P, 8], dt.float32)
    b2 = din("b2", [P, 8], dt.float32)
    ident_d = din("ident", [P, P], dt.bfloat16)
    maskm_d = din("maskm", [P, P], dt.bfloat16)
    jidx_d = din("jidx", [1, 1], dt.uint32)
    out_d = nc.dram_tensor("out", [P, 8, TOK], dt.float32, kind="ExternalOutput").ap()

    VW = 66  # per-head V row width: 64 v + ones col (64) + pad (65)

    with tile.TileContext(nc) as tc:
        from contextlib import ExitStack

        ctx = ExitStack()
        with ctx:
            c_pool = ctx.enter_context(tc.tile_pool(name="consts", bufs=1))
            dram = ctx.enter_context(tc.tile_pool(name="dram", bufs=1, space="DRAM"))

            # ---- per-core j register (for Switch) ----
            jreg = nc.alloc_registers(
                "jreg",
                [mybir.EngineType.PE, mybir.EngineType.Activation,
                 mybir.EngineType.DVE, mybir.EngineType.SP],
            )
            nc.regs_load(jreg, jidx_d[0:1, 0:1])
            jval = nc.snap(jreg, donate=True, min_val=0, max_val=3)

            # ---- constants ----
            ident = c_pool.tile([P, P], dt.bfloat16, tag="ident")
            nc.sync.dma_start(ident[:], ident_d[:])
            maskm = c_pool.tile([P, P], dt.bfloat16, tag="maskm")
            nc.sync.dma_start(maskm[:], maskm_d[:])
            ones128_bf = c_pool.tile([P, 1], dt.bfloat16, tag="ones128")
            nc.vector.memset(ones128_bf[:], 1.0)
            ones1_bf = c_pool.tile([1, P], dt.bfloat16, tag="ones1b")
            nc.vector.memset(ones1_bf[:], 1.0)
            ones_hi = c_pool.tile([65, 64], dt.bfloat16, tag="oneshi")
            nc.vector.memset(ones_hi[64:65, :], 1.0)

            bqk_sb = c_pool.tile([P, 16], dt.float32, tag="bqk")
            nc.sync.dma_start(bqk_sb[:], b_qk[:])
            bo_sb = c_pool.tile([P, 8], dt.float32, tag="bo")
            nc.sync.dma_start(bo_sb[:], b_o[:])
            bfc_sb = c_pool.tile([P, 32], dt.float32, tag="bfc")
            nc.sync.dma_start(bfc_sb[:], b_fc[:])
            bpr_sb = c_pool.tile([P, 8], dt.float32, tag="bpr")
            nc.sync.dma_start(bpr_sb[:], b_pr[:])
            g1_sb = c_pool.tile([P, 8], dt.float32, tag="g1")
            nc.sync.dma_start(g1_sb[:], g1[:])
            b1_sb = c_pool.tile([P, 8], dt.float32, tag="b1")
            nc.sync.dma_start(b1_sb[:], b1[:])
            g2_sb = c_pool.tile([P, 8], dt.float32, tag="g2")
            nc.sync.dma_start(g2_sb[:], g2[:])
            b2_sb = c_pool.tile([P, 8], dt.float32, tag="b2")
            nc.sync.dma_start(b2_sb[:], b2[:])

            # ---- AG buffers: 4 fp8 collectives, order k0,v0,v1,k1 ----
            KH = 4 * P * TOK                 # half of kT  [4,128,512]
            VH = 2 * P * (16 * VW)           # half of v   [2,128,1056]
            ag_k_in = [dram.tile([KH], dt.float8e4, tag=f"agki{i}", name=f"agki{i}") for i in range(2)]
            ag_k_out = [dram.tile([4, KH], dt.float8e4, tag=f"agko{i}", name=f"agko{i}") for i in range(2)]
            ag_v_in = [dram.tile([VH], dt.float8e4, tag=f"agvi{i}", name=f"agvi{i}") for i in range(2)]
            ag_v_out = [dram.tile([4, VH], dt.float8e4, tag=f"agvo{i}", name=f"agvo{i}") for i in range(2)]
            GROUPS = [[0, 1, 2, 3], [4, 5, 6, 7]]

            # ---- residual/MLP tiles live from phase 4 to the end ----
            pN_cm = tc.tile_pool(name="pN", bufs=1)
            pN = pN_cm.__enter__()
            r1_bf = pN.tile([P, 8, TOK], dt.bfloat16, tag="r1bf")
            n_bf = pN.tile([P, 8, TOK], dt.bfloat16, tag="n_bf")
            h1g = pN.tile([P, 32, TOK], dt.float8e4, tag="h1g")
            r2_bf = pN.tile([P, 8, TOK], dt.bfloat16, tag="r2bf")
            wfc0 = pN.tile([P, 8, 1024], dt.bfloat16, tag="wfc0")

            # ---- qT / o_cat / wo live from phase 1 through w_o ----
            pA_cm = tc.tile_pool(name="pA", bufs=1)
            pA = pA_cm.__enter__()
            qT = pA.tile([P, 8, TOK], dt.float8e4, tag="qT")
            o_cat = pA.tile([P, 8, TOK], dt.float8e4, tag="o_cat")
            o_tmp = [pA.tile([64, TOK], dt.float8e4, tag=f"o_tmp{i}", name=f"o_tmp{i}") for i in range(NHP)]
            wo8 = pA.tile([P, 4, 2, D], dt.float8e4, tag="wo8")
            xfs_bf = pA.tile([P, 8, TOK], dt.bfloat16, tag="xfs")

            # gathered K/V + own-kv pools open across phase1+attention
            gat_cm = tc.tile_pool(name="gathered", bufs=1)
            gat = gat_cm.__enter__()
            kT_pack = gat.tile([P, 8, S], dt.float8e4, tag="kT_pack")
            v_ones = gat.tile([P, 16, 16 * VW], dt.float8e4, tag="v_ones")
            kvp_cm = tc.tile_pool(name="kv_own", bufs=1)
            kvp = kvp_cm.__enter__()
            v_own = kvp.tile([P, 4, 16 * VW], dt.float8e4, tag="v_own")
            kT_own = kvp.tile([P, 8, TOK], dt.float8e4, tag="kT_own")

            # ============ phase 1: qkv (fp8 DoubleRow) ============
            with (
                tc.tile_pool(name="xw", bufs=2) as xw,
                tc.tile_pool(name="qkv_ps", bufs=4, space="PSUM") as qkv_ps,
            ):
                xb8 = xw.tile([P, 4, 2, TOK], dt.float8e4, tag="xb8")
                nc.sync.dma_start(xb8[:], x8_d[:])
                wq2 = xw.tile([P, 4, 2, 512], dt.float8e4, tag="wqkq", name="wq2")
                nc.sync.dma_start(wq2[:], w_qk[2])
                wv = xw.tile([P, 4, 2, D], dt.float8e4, tag="wv")
                nc.sync.dma_start(wv[:], w_v[:])

                v4 = v_own.rearrange("p c (h w) -> p c h w", w=VW)
                nc.vector.memset(v4[:, :, :, 64:66], 0.0)
                nc.vector.memset(v4[:, :, :, 64:65], 1.0)

                def v_block(t4):
                    for vc in range(2):
                        ps = qkv_ps.tile([P, 512], dt.float32, tag="qkvps")
                        for kc2 in range(4):
                            nc.tensor.matmul(
                                ps[:],
                                lhsT=xb8[:, kc2, :, t4 * P:(t4 + 1) * P],
                                rhs=wv[:, kc2, :, vc * 512:(vc + 1) * 512],
                                start=(kc2 == 0),
                                stop=(kc2 == 3),
                                perf_mode=DR,
                            )
                        dest = v4[:, t4, vc * 8:(vc + 1) * 8, 0:64]
                        nc.vector.tensor_scalar(
                            dest, ps[:].rearrange("p (h w) -> p h w", w=64),
                            1.0 / WSC, None, OP.mult,
                        )

                def ship_v(i):
                    nc.sync.dma_start(
                        ag_v_in[i].rearrange("(c p f) -> p c f", c=2, p=P),
                        v_own[:, 2 * i:2 * i + 2, :],
                    )
                    nc.gpsimd.collective_compute(
                        "AllGather", mybir.AluOpType.bypass,
                        replica_groups=GROUPS,
                        ins=[ag_v_in[i].opt()], outs=[ag_v_out[i].opt()],
                    )

                def ship_k(i):
                    nc.sync.dma_start(
                        ag_k_in[i].rearrange("(c p t) -> p c t", c=4, p=P),
                        kT_own[:, 4 * i:4 * i + 4, :],
                    )
                    nc.gpsimd.collective_compute(
                        "AllGather", mybir.AluOpType.bypass,
                        replica_groups=GROUPS,
                        ins=[ag_k_in[i].opt()], outs=[ag_k_out[i].opt()],
                    )

                def qk_quarter(qi, wq):
                    # w_qk columns: [q(0:1024) | k(1024:2048)]
                    for fo in range(4):
                        fchunk = 4 * qi + fo
                        ps = qkv_ps.tile([P, 512], dt.float32, tag="qkvps")
                        for kc2 in range(4):
                            nc.tensor.matmul(
                                ps[:],
                                lhsT=wq[:, kc2, :, fo * P:(fo + 1) * P],
                                rhs=xb8[:, kc2, :, :],
                                start=(kc2 == 0),
                                stop=(kc2 == 3),
                                perf_mode=DR,
                            )
                        dest = kT_own[:, fchunk - 8, :] if fchunk >= 8 else qT[:, fchunk, :]
                        nc.scalar.activation(
                            dest, ps[:], AF.Identity,
                            bias=bqk_sb[:, fchunk:fchunk + 1], scale=1.0 / WSC,
                        )

                # order: kq2 -> AGk0; v01 -> AGv0; v23 -> AGv1; kq3 -> AGk1;
                # then q quarters (only needed by local attention pass 1).
                qk_quarter(2, wq2)
                ship_k(0)
                wq3 = xw.tile([P, 4, 2, 512], dt.float8e4, tag="wqkq", name="wq3")
                nc.sync.dma_start(wq3[:], w_qk[3])
                v_block(0)
                v_block(1)
                ship_v(0)
                wq0 = xw.tile([P, 4, 2, 512], dt.float8e4, tag="wqkq", name="wq0")
                nc.sync.dma_start(wq0[:], w_qk[0])
                v_block(2)
                v_block(3)
                ship_v(1)
                qk_quarter(3, wq3)
                ship_k(1)
                wq1 = xw.tile([P, 4, 2, 512], dt.float8e4, tag="wqkq", name="wq1")
                nc.sync.dma_start(wq1[:], w_qk[1])
                qk_quarter(0, wq0)
                qk_quarter(1, wq1)

            # prefetch (DMA engines, overlaps attention)
            nc.sync.dma_start(wo8[:], w_o[:])
            nc.sync.dma_start(xfs_bf[:], xT_bf[:])
            nc.sync.dma_start(wfc0[:], w_fc[0])

            # ============ phase 2+3: K/V load + attention ============
            with (
                tc.tile_pool(name="att_s", bufs=2, space="PSUM") as s_ps_pool,
                tc.tile_pool(name="att_o", bufs=1, space="PSUM") as o_ps_pool,
                tc.tile_pool(name="att_rep", bufs=1, space="PSUM") as rep_ps_pool,
                tc.tile_pool(name="att_tmp", bufs=3) as atmp,
                tc.tile_pool(name="att_tmp2", bufs=2) as atmp2,
                tc.tile_pool(name="att_loc", bufs=1) as alocp,
            ):
                def _load_k(r, i, j):
                    if r == j:
                        kk = kT_own[:, 4 * i:4 * i + 4, :]
                    else:
                        kk = ag_k_out[i][r].rearrange("(c p t) -> p c t", c=4, p=P)
                    for hc in range(4):
                        hp = 4 * i + hc
                        nc.sync.dma_start(
                            kT_pack[:, hp, r * 256:(r + 1) * 256], kk[:, hc, 0:256])
                        nc.sync.dma_start(
                            kT_pack[:, hp, (7 - r) * 256:(8 - r) * 256],
                            kk[:, hc, 256:512])

                def _load_v(r, i, j):
                    if r == j:
                        vv = v_own[:, 2 * i:2 * i + 2, :]
                    else:
                        vv = ag_v_out[i][r].rearrange("(c p f) -> p c f", c=2, p=P)
                    for lc in range(2):
                        lt = 2 * i + lc
                        gb = r if lt < 2 else 7 - r
                        g16 = 2 * gb + (lt % 2)
                        nc.sync.dma_start(v_ones[:, g16, :], vv[:, lc, :])

                def load_kv(j):
                    """Fill kT_pack / v_ones.  Own rank straight from SBUF;
                    remote halves in AG completion order (k0, v0, v1, k1)."""
                    for i in range(2):
                        _load_k(j, i, j)
                        _load_v(j, i, j)
                    rem = [r for r in range(4) if r != j]
                    for r in rem:
                        _load_k(r, 0, j)
                    for r in rem:
                        _load_v(r, 0, j)
                    for r in rem:
                        _load_v(r, 1, j)
                    for r in rem:
                        _load_k(r, 1, j)

                # ====== attention: QK+exp now, AV deferred one chunk so the
                # in-order PE queue never stalls on exp ======
                def attn_qk_exp(hp, kc, gq):
                    vi = [i for i in range(4) if gq[i] >= kc]
                    qs = P * vi[0]
                    diag = gq[vi[0]] == kc
                    sps = s_ps_pool.tile([P, 2, TOK], dt.float32, tag="sps")
                    for h2 in range(2):
                        lo, hi = 64 * h2, 64 * (h2 + 1)
                        nc.tensor.matmul(
                            sps[:, h2, qs:],
                            lhsT=kT_pack[lo:hi, hp, kc * P:(kc + 1) * P],
                            rhs=qT[lo:hi, hp, qs:],
                            start=True,
                            stop=not diag,
                            tile_position=(64 * h2, 0),
                        )
                        if diag:
                            nc.tensor.matmul(
                                sps[:, h2, qs:qs + P],
                                lhsT=ident[:],
                                rhs=maskm[:],
                                start=False,
                                stop=True,
                            )
                    aT = atmp.tile([P, 2, TOK], dt.float8e4, tag="aT")
                    nc.scalar.activation(
                        aT[:, :, qs:], sps[:, :, qs:], AF.Exp, scale=0.125
                    )
                    return (hp, kc, qs, aT)

                def attn_av(oT, pend, first, last):
                    hp, kc, qs, aT = pend
                    for h2 in range(2):
                        h = 2 * hp + h2
                        nc.tensor.matmul(
                            oT[:, h2, qs:],
                            lhsT=v_ones[:, kc, h * VW:h * VW + 65],
                            rhs=aT[:, h2, qs:],
                            start=first,
                            stop=last,
                        )

                def attn_chunks(hp, chunks, gq, oT):
                    pend = None
                    for ki, kc in enumerate(chunks):
                        cur = attn_qk_exp(hp, kc, gq)
                        if pend is not None:
                            attn_av(oT, pend, ki == 1, False)
                        pend = cur
                    attn_av(oT, pend, len(chunks) == 1, True)

                def attn_arm(j):
                    load_kv(j)
                    gq = _q_blocks(j)
                    last_kc = gq[-1]
                    remote = [k for k in range(last_kc + 1) if k not in gq]
                    # pass 1: all local chunks (overlap the AllGather chain)
                    oloc = []
                    for hp in range(NHP):
                        oT = o_ps_pool.tile([65, 2, TOK], dt.float32, tag="oT")
                        attn_chunks(hp, gq, gq, oT)
                        ol = alocp.tile([65, 2, TOK], dt.float32,
                                       tag=f"oloc{hp}", name=f"ol{hp}")
                        nc.vector.tensor_copy(ol[:], oT[:])
                        oloc.append(ol)
                    # pass 2: remote chunks + local partial + normalize
                    qs0 = P * [i for i in range(4) if gq[i] >= remote[0]][0]
                    for hp in range(NHP):
                        oT = o_ps_pool.tile([65, 2, TOK], dt.float32, tag="oT")
                        attn_chunks(hp, remote, gq, oT)
                        osum = atmp2.tile([65, 2, TOK], dt.float32, tag="osum")
                        if qs0 > 0:
                            nc.vector.tensor_copy(osum[:, :, 0:qs0], oloc[hp][:, :, 0:qs0])
                        nc.vector.tensor_tensor(
                            osum[:, :, qs0:], oT[:, :, qs0:], oloc[hp][:, :, qs0:], OP.add)
                        # denominator row -> bf16 -> PE broadcast -> DVE recip
                        rrow = atmp2.tile([65, 2, TOK], dt.bfloat16, tag="rrow")
                        nc.vector.tensor_copy(rrow[64:65, :, :], osum[64:65, :, :])
                        rep = rep_ps_pool.tile([64, 2, TOK], dt.float32, tag="rep")
                        for h2 in range(2):
                            nc.tensor.matmul(
                                rep[:, h2, :],
                                lhsT=ones_hi[64:65, :],
                                rhs=rrow[64:65, h2, :],
                                start=True,
                                stop=True,
                            )
                        rrec = atmp2.tile([64, 2, TOK], dt.float32, tag="rrec")
                        nc.vector.reciprocal_approx_fast(out=rrec[:], in_=rep[:])
                        for h2 in range(2):
                            dest = o_cat[0:64, hp, :] if h2 == 0 else o_tmp[hp][:]
                            nc.vector.tensor_tensor(
                                dest, osum[0:64, h2, :], rrec[:, h2, :], OP.mult
                            )
                        nc.sync.dma_start(o_cat[64:128, hp, :], o_tmp[hp][:])

                for case in tc.Switch(jval, 4):
                    attn_arm(case)

            kvp_cm.__exit__(None, None, None)
            gat_cm.__exit__(None, None, None)

            # ============ layernorm finalize helper (stats already in psum) ====
            def ln_finalize(lnp, lnrep, s1, s2, tag):
                mu = lnp.tile([1, TOK], dt.float32, tag=f"mu_{tag}")
                nc.vector.tensor_scalar(mu[:], s1[:], 1.0 / D, None, OP.mult)
                m2 = lnp.tile([1, TOK], dt.float32, tag=f"m2_{tag}")
                nc.vector.tensor_scalar(m2[:], s2[:], 1.0 / D, LN_EPS, OP.mult, OP.add)
                var = lnp.tile([1, TOK], dt.float32, tag=f"var_{tag}")
                nc.vector.tensor_tensor(var[:], mu[:], mu[:], OP.mult)
                nc.vector.tensor_tensor(var[:], m2[:], var[:], OP.subtract)
                lnv = lnp.tile([1, TOK], dt.float32, tag=f"lnv_{tag}")
                nc.scalar.activation(lnv[:], var[:], AF.Ln)
                rsmu = lnp.tile([1, 2, TOK], dt.bfloat16, tag=f"rsmu_{tag}")
                nc.scalar.activation(rsmu[:, 0, :], lnv[:], AF.Exp, scale=-0.5)
                nc.vector.tensor_tensor(
                    rsmu[:, 1, :], mu[:], rsmu[:, 0, :], OP.mult
                )
                rep = lnrep.tile([P, 2, TOK], dt.float32, tag=f"rep_{tag}")
                for i in range(2):
                    nc.tensor.matmul(
                        rep[:, i, :], lhsT=ones1_bf[:], rhs=rsmu[:, i, :],
                        start=True, stop=True,
                    )
                return rep

            # ============ phase 4: w_o (fp8 DR) + residual + LN1 ============
            with (
                tc.tile_pool(name="wo_ps", bufs=4, space="PSUM") as wo_ps,
                tc.tile_pool(name="ln1_s", bufs=1, space="PSUM") as ln1_s,
                tc.tile_pool(name="ln1_t", bufs=2) as ln1_t,
                tc.tile_pool(name="ln1_f", bufs=1) as ln1_f,
                tc.tile_pool(name="ln1_r", bufs=1, space="PSUM") as ln1_r,
            ):
                s1 = ln1_s.tile([1, TOK], dt.float32, tag="s1")
                s2 = ln1_s.tile([1, TOK], dt.float32, tag="s2")
                for oc in range(8):
                    ps = wo_ps.tile([P, 512], dt.float32, tag="wops")
                    for kc2 in range(4):
                        nc.tensor.matmul(
                            ps[:],
                            lhsT=wo8[:, kc2, :, oc * P:(oc + 1) * P],
                            rhs=o_cat[:, 2 * kc2:2 * kc2 + 2, :],
                            start=(kc2 == 0),
                            stop=(kc2 == 3),
                            perf_mode=DR,
                        )
                    t0 = ln1_t.tile([P, TOK], dt.float32, tag="t0")
                    nc.vector.tensor_scalar(
                        t0[:], ps[:], 1.0 / WSC, bo_sb[:, oc:oc + 1], OP.mult, OP.add
                    )
                    nc.vector.tensor_tensor(r1_bf[:, oc, :], t0[:], xfs_bf[:, oc, :], OP.add)
                    sq = ln1_t.tile([P, TOK], dt.bfloat16, tag="sq1")
                    nc.vector.tensor_tensor(sq[:], r1_bf[:, oc, :], r1_bf[:, oc, :], OP.mult)
                    nc.tensor.matmul(
                        s1[:], lhsT=ones128_bf[:], rhs=r1_bf[:, oc, :],
                        start=(oc == 0), stop=(oc == 7),
                    )
                    nc.tensor.matmul(
                        s2[:], lhsT=ones128_bf[:], rhs=sq[:],
                        start=(oc == 0), stop=(oc == 7),
                    )
                rep1 = ln_finalize(ln1_f, ln1_r, s1, s2, "ln1")
                for c8 in range(8):
                    t = ln1_t.tile([P, TOK], dt.float32, tag="t1")
                    nc.vector.tensor_tensor(
                        t[:], r1_bf[:, c8, :], rep1[:, 0, :], OP.mult
                    )
                    nc.vector.tensor_tensor(t[:], t[:], rep1[:, 1, :], OP.subtract)
                    nc.vector.tensor_scalar(
                        n_bf[:, c8, :], t[:], g1_sb[:, c8:c8 + 1], b1_sb[:, c8:c8 + 1],
                        OP.mult, OP.add,
                    )

            pA_cm.__exit__(None, None, None)

            # ============ phase 5: MLP (fc bf16, pr fp8 DR) ============
            ln2_s_cm = tc.tile_pool(name="ln2_s", bufs=1, space="PSUM")
            ln2_s = ln2_s_cm.__enter__()
            s1b = ln2_s.tile([1, TOK], dt.float32, tag="s1b")
            s2b = ln2_s.tile([1, TOK], dt.float32, tag="s2b")
            wprp_cm = tc.tile_pool(name="wpr", bufs=1)
            wprp = wprp_cm.__enter__()
            wpr_sb = wprp.tile([P, 4, 4, 2, 1024], dt.float8e4, tag="wprq")
            nc.sync.dma_start(wpr_sb[:, 0], w_pr[0])

            with (
                tc.tile_pool(name="wfc", bufs=2) as wfcp,
                tc.tile_pool(name="fc_ps", bufs=4, space="PSUM") as fc_ps,
            ):
                for oq in range(4):
                    if oq == 0:
                        wq = wfc0
                    else:
                        wq = wfcp.tile([P, 8, 1024], dt.bfloat16, tag="wfcq")
                        nc.sync.dma_start(wq[:], w_fc[oq])
                        nc.sync.dma_start(wpr_sb[:, oq], w_pr[oq])
                    for oc8 in range(8):
                        oc = oq * 8 + oc8
                        ps = fc_ps.tile([P, 512], dt.float32, tag="fcps")
                        for c8 in range(8):
                            nc.tensor.matmul(
                                ps[:],
                                lhsT=wq[:, c8, oc8 * P:(oc8 + 1) * P],
                                rhs=n_bf[:, c8, :],
                                start=(c8 == 0),
                                stop=(c8 == 7),
                            )
                        nc.scalar.activation(
                            h1g[:, oc, :], ps[:], AF.Gelu,
                            bias=bfc_sb[:, oc:oc + 1],
                        )

            with (
                tc.tile_pool(name="pr_ps", bufs=2, space="PSUM") as pr_ps,
                tc.tile_pool(name="ln2_t", bufs=2) as ln2_t,
            ):
                for oc in range(8):
                    ps = pr_ps.tile([P, 512], dt.float32, tag="prps")
                    for q4 in range(4):
                        for kc2 in range(4):
                            nc.tensor.matmul(
                                ps[:],
                                lhsT=wpr_sb[:, q4, kc2, :, oc * P:(oc + 1) * P],
                                rhs=h1g[:, 8 * q4 + 2 * kc2:8 * q4 + 2 * kc2 + 2, :],
                                start=(q4 == 0 and kc2 == 0),
                                stop=(q4 == 3 and kc2 == 3),
                                perf_mode=DR,
                            )
                    t = ln2_t.tile([P, TOK], dt.float32, tag="tm")
                    nc.vector.tensor_scalar(
                        t[:], ps[:], 1.0 / WSC, bpr_sb[:, oc:oc + 1], OP.mult, OP.add
                    )
                    nc.vector.tensor_tensor(r2_bf[:, oc, :], t[:], n_bf[:, oc, :], OP.add)
                    sq = ln2_t.tile([P, TOK], dt.bfloat16, tag="sq2")
                    nc.vector.tensor_tensor(sq[:], r2_bf[:, oc, :], r2_bf[:, oc, :], OP.mult)
                    nc.tensor.matmul(
                        s1b[:], lhsT=ones128_bf[:], rhs=r2_bf[:, oc, :],
                        start=(oc == 0), stop=(oc == 7),
                    )
                    nc.tensor.matmul(
                        s2b[:], lhsT=ones128_bf[:], rhs=sq[:],
                        start=(oc == 0), stop=(oc == 7),
                    )
            wprp_cm.__exit__(None, None, None)

            with (
                tc.tile_pool(name="ln2_f", bufs=1) as ln2_f,
                tc.tile_pool(name="ln2_r", bufs=1, space="PSUM") as ln2_r,
                tc.tile_pool(name="ln2_o", bufs=2) as ln2_o,
            ):
                rep2 = ln_finalize(ln2_f, ln2_r, s1b, s2b, "ln2")
                for c8 in range(8):
                    t = ln2_o.tile([P, TOK], dt.float32, tag="t2")
                    nc.vector.tensor_tensor(
                        t[:], r2_bf[:, c8, :], rep2[:, 0, :], OP.mult
                    )
                    nc.vector.tensor_tensor(t[:], t[:], rep2[:, 1, :], OP.subtract)
                    h = ln2_o.tile([P, TOK], dt.float32, tag="h2o")
                    nc.vector.tensor_scalar(
                        h[:], t[:], g2_sb[:, c8:c8 + 1], b2_sb[:, c8:c8 + 1],
                        OP.mult, OP.add,
                    )
                    nc.sync.dma_start(out_d[:, c8, :], h[:])
            ln2_s_cm.__exit__(None, None, None)
            pN_cm.__exit__(None, None, None)

    nc.compile()
    return nc


def _prep_shared(w_attn, b_attn, w_o, b_o, ln1_g, ln1_b, w_fc, b_fc, w_pr, b_pr,
                 ln2_g, ln2_b):
    w_attn = np.asarray(w_attn, np.float32)
    b_attn = np.asarray(b_attn, np.float32)
    w_o_f = np.asarray(w_o, np.float32)
    b_v = b_attn[2 * D:]
    b_o_eff = (np.asarray(b_o, np.float32) + b_v @ w_o_f).astype(np.float32)
    mask = np.where(
        np.arange(P)[:, None] > np.arange(P)[None, :], MASK_NEG, 0.0
    ).astype(BF16)  # [ki, qj]: mask keys above the diagonal

    def pc8(w):  # [D, X] -> [128, 8, X] partition-major
        return np.ascontiguousarray(
            w.reshape(8, P, -1).transpose(1, 0, 2))

    def pdr(w):  # [1024, X] (contraction) -> [128, 4, 2, X] DoubleRow fp8
        return np.ascontiguousarray(
            (w * WSC).reshape(4, 2, P, -1).transpose(2, 0, 1, 3)).astype(FP8)

    def col(b, n):  # [n*128] -> [128, n]
        return np.ascontiguousarray(b.reshape(n, P).T)

    w_qk_q = np.stack([pdr(w_attn[:, 512 * qi:512 * (qi + 1)]) for qi in range(4)])
    wfc = np.asarray(w_fc, np.float32)
    wpr = np.asarray(w_pr, np.float32)
    w_fc_q = np.stack([pc8(wfc[:, 1024 * oq:1024 * (oq + 1)]).astype(BF16)
                       for oq in range(4)])
    w_pr_q = np.stack([pdr(wpr[1024 * q4:1024 * (q4 + 1), :]) for q4 in range(4)])

    shared = {
        "w_qk": w_qk_q,
        "w_v": pdr(w_attn[:, 2 * D:]),
        "w_o": pdr(w_o_f),
        "w_fc": w_fc_q,
        "w_pr": w_pr_q,
        "b_qk": col(b_attn[: 2 * D], 16),
        "b_o": col(b_o_eff, 8),
        "b_fc": col(np.asarray(b_fc, np.float32), 32),
        "b_pr": col(np.asarray(b_pr, np.float32), 8),
        "g1": col(np.asarray(ln1_g, np.float32), 8),
        "b1": col(np.asarray(ln1_b, np.float32), 8),
        "g2": col(np.asarray(ln2_g, np.float32), 8),
        "b2": col(np.asarray(ln2_b, np.float32), 8),
        "ident": np.eye(P, dtype=np.float32).astype(BF16),
        "maskm": mask,
    }
    return shared


def kernel(x, w_attn, b_attn, w_o, b_o, ln1_g, ln1_b, w_fc, b_fc, w_pr, b_pr,
           ln2_g, ln2_b, _trace=False, _tmpdir=None):
    from concourse.bass_utils import run_bass_kernel_spmd

    if "nc" not in _CACHE:
        _CACHE["nc"] = _build()
    nc = _CACHE["nc"]

    x = np.asarray(x, np.float32)
    shared = _prep_shared(w_attn, b_attn, w_o, b_o, ln1_g, ln1_b, w_fc, b_fc,
                          w_pr, b_pr, ln2_g, ln2_b)

    in_maps = []
    idxs = []
    for c in range(N_CORES):
        b, j = c // 4, c % 4
        idx = np.r_[j * 256:(j + 1) * 256, (7 - j) * 256:(8 - j) * 256]
        idxs.append((b, idx))
        xT = x[b, idx, :].T  # [D, TOK]
        xb = np.ascontiguousarray(xT.reshape(8, P, TOK).transpose(1, 0, 2)).astype(BF16)
        x8 = np.ascontiguousarray(
            xb.astype(np.float32).reshape(P, 4, 2, TOK)).astype(FP8)
        m = dict(shared)
        m["xT_bf"] = xb
        m["x8"] = x8
        m["jidx"] = np.array([[j]], np.uint32)
        in_maps.append(m)

    res = run_bass_kernel_spmd(
        nc, in_maps, core_ids=list(range(N_CORES)), trace=_trace, tmpdir=_tmpdir
    )
    if _trace:
        _CACHE["exec_time_ns"] = res.exec_time_ns

    out = np.empty((B, S, D), np.float32)
    for c in range(N_CORES):
        b, idx = idxs[c]
        o = np.asarray(res.results[c]["out"], np.float32)  # [128, 8, 512]
        out[b, idx, :] = o.transpose(1, 0, 2).reshape(D, TOK).T
    return out
